# revision 8
# baseline (speedup 1.0000x reference)
"""Multi-head causal attention (B=2, T=2048, C=2048, 16 heads, fp32) on 8
Trainium2 NeuronCores.

Sharding: data-parallel over batch (2) x tensor-parallel over heads
(4 heads/core).  Core c handles batch c//4, heads 4*(c%4)..4*(c%4)+3.
Each core computes q/k/v projections for its heads, causal softmax
attention, and a partial output projection (its heads' rows of Wout);
the host sums the 4 partials per batch.

All big matmuls run in float32r (fp32 with 11-bit mantissa): products of
fp32r values accumulate exactly in fp32 PSUM, so the only precision loss
is the input rounding (~1.2e-4 relative).

Device program per core:
  Phase A: stream x^T in 256-wide t-slabs against resident Wqk/Wv.
    q^T,k^T (head-dim on partitions) bounce through DRAM scratch tiles
    (one per (row, tq-block) for fine-grained deps); v ([t,d] layout)
    stays resident in SBUF as one tile per t-tile.
  Phase D, per 512-wide tq-block b, per head:
    for each tk-tile j<=diag: scoresT[tk,tq] = kT_j^T @ qT_b on PE;
    additive -1e30 causal mask on diagonal-crossing tiles (DVE, static
    mask tiles built once via affine_select); exp via ACT
    (scale=1/sqrt(128) folded in) -> attnT_j in fp32r; then
      outT[d,tq]   += v_j^T   @ attnT_j   (n=512, full-rate fp32r)
      denom[1,tq]  += ones^T  @ attnT_j   (softmax denominators)
    normalize: DVE reciprocal of denom -> broadcast over partitions via
    a K=1 fp32 matmul -> attn_outT = outT * recip (DVE), directly in the
    [d, tq] layout the output projection needs (no transposes).
  Output projection per block: out[t,c] += attn_outT_h^T @ Wout_h.
"""

import numpy as np

import concourse.bass as bass
import concourse.tile as tile
from concourse import bacc, mybir
from concourse.bass_utils import run_bass_kernel_spmd

B, T, C = 2, 2048, 2048
H, DH = 16, 128
HPC = 4            # heads per core
KO = C // 128      # 16 contraction tiles
NSLAB = 8          # 256-wide t slabs in phase A
SLAB = T // NSLAB  # 256
NB = 4             # 512-wide tq blocks in phase D
BW = T // NB       # 512
NT = T // 128      # 16 t tiles
SCALE = DH ** -0.5
F32 = mybir.dt.float32
F32R = mybir.dt.float32r


def round_fp32r(x: np.ndarray) -> np.ndarray:
    """Round fp32 to fp32r (11-bit mantissa) the way the PE expects."""
    u = x.astype(np.float32, copy=True).view(np.uint32).astype(np.uint64)
    u = ((u + 0x1000) >> 13) << 13
    return (u & 0xFFFFFFFF).astype(np.uint32).view(np.float32)


def build_nc():
    nc = bacc.Bacc("TRN2", target_bir_lowering=False, debug=False, num_devices=8)
    xt_d = nc.dram_tensor("xt", [C, T], F32R, kind="ExternalInput")
    wqk_d = nc.dram_tensor("wqk", [C, 2 * HPC * DH], F32R, kind="ExternalInput")
    wv_d = nc.dram_tensor("wv", [C, HPC * DH], F32R, kind="ExternalInput")
    wout_d = nc.dram_tensor("wout", [HPC * DH, C], F32R, kind="ExternalInput")
    out_d = nc.dram_tensor("out", [T, C], F32, kind="ExternalOutput")

    xt = xt_d.ap().rearrange("(ko p) t -> p ko t", p=128)
    wqk = wqk_d.ap().rearrange("(ko p) m -> p ko m", p=128)
    wv = wv_d.ap().rearrange("(ko p) m -> p ko m", p=128)
    wout = wout_d.ap().rearrange("(h p) c -> p h c", p=128)
    out = out_d.ap()

    with tile.TileContext(nc) as tc:
        from contextlib import ExitStack

        with ExitStack() as top:
            # ---- persistent across phases ----
            vp_pool = top.enter_context(tc.tile_pool(name="vp", bufs=NT))
            dram_pool = top.enter_context(tc.tile_pool(name="dram", bufs=1, space="DRAM"))

            # v resident, one tile per t-tile: [tk within tile, head, d]
            vp = [vp_pool.tile([128, HPC, DH], F32R, name=f"vp{j}", tag="vp")
                  for j in range(NT)]
            # q^T/k^T DRAM scratch, one tile per (row co, tq block):
            # co 0..3 = q heads, 4..7 = k heads
            qk_dr = [[dram_pool.tile([128, BW], F32R, name=f"qk{co}_{b}")
                      for b in range(NB)] for co in range(2 * HPC)]

            # ================= Phase A: projections =================
            with ExitStack() as pa:
                wqk_pool = pa.enter_context(tc.tile_pool(name="wqk", bufs=1))
                wv_pool = pa.enter_context(tc.tile_pool(name="wv", bufs=1))
                slab_pool = pa.enter_context(tc.tile_pool(name="slab", bufs=2))
                bounce_pool = pa.enter_context(tc.tile_pool(name="bounce", bufs=3))
                psa_qk = pa.enter_context(tc.tile_pool(name="psa_qk", bufs=2, space="PSUM"))
                psa_v = pa.enter_context(tc.tile_pool(name="psa_v", bufs=2, space="PSUM"))

                wqk_sb = wqk_pool.tile([128, KO, 2 * HPC * DH], F32R)
                wv_sb = wv_pool.tile([128, KO, HPC * DH], F32R)
                for ko in range(KO):
                    nc.sync.dma_start(wqk_sb[:, ko], wqk[:, ko])

                for s in range(NSLAB):
                    slab = slab_pool.tile([128, KO, SLAB], F32R)
                    for ko in range(KO):
                        nc.sync.dma_start(slab[:, ko], xt[:, ko, s * SLAB:(s + 1) * SLAB])
                    if s == 0:
                        for ko in range(KO):
                            nc.sync.dma_start(wv_sb[:, ko], wv[:, ko])
                    for co in range(2 * HPC):
                        ps = psa_qk.tile([128, SLAB], F32)
                        for ko in range(KO):
                            nc.tensor.matmul(
                                ps[:], wqk_sb[:, ko, co * 128:(co + 1) * 128],
                                slab[:, ko], start=(ko == 0), stop=(ko == KO - 1),
                            )
                        bt = bounce_pool.tile([128, SLAB], F32R)
                        nc.vector.tensor_copy(bt[:], ps[:])
                        blk, off = (s * SLAB) // BW, (s * SLAB) % BW
                        nc.sync.dma_start(qk_dr[co][blk][:, off:off + SLAB], bt[:])
                    for tt in range(SLAB // 128):
                        ps = psa_v.tile([128, HPC * DH], F32)
                        for ko in range(KO):
                            nc.tensor.matmul(
                                ps[:], slab[:, ko, tt * 128:(tt + 1) * 128],
                                wv_sb[:, ko], start=(ko == 0), stop=(ko == KO - 1),
                            )
                        ti = s * (SLAB // 128) + tt
                        for h in range(HPC):
                            nc.vector.tensor_copy(vp[ti][:, h], ps[:, h * 128:(h + 1) * 128])

            # ============ Phase D: attention + out projection ============
            with ExitStack() as pd:
                const_pool = pd.enter_context(tc.tile_pool(name="const", bufs=1))
                wout_pool = pd.enter_context(tc.tile_pool(name="wout", bufs=1))
                kt_pool = pd.enter_context(tc.tile_pool(name="kt", bufs=4))
                qt_pool = pd.enter_context(tc.tile_pool(name="qt", bufs=2))
                at_pool = pd.enter_context(tc.tile_pool(name="at", bufs=3))
                rec_pool = pd.enter_context(tc.tile_pool(name="rec", bufs=2))
                bc_pool = pd.enter_context(tc.tile_pool(name="bc", bufs=2))
                aot_pool = pd.enter_context(tc.tile_pool(name="aot", bufs=2))
                oc_pool = pd.enter_context(tc.tile_pool(name="oc", bufs=3))
                psd_s = pd.enter_context(tc.tile_pool(name="psd_s", bufs=2, space="PSUM"))
                psd_o = pd.enter_context(tc.tile_pool(name="psd_o", bufs=2, space="PSUM"))
                psd_n = pd.enter_context(tc.tile_pool(name="psd_n", bufs=2, space="PSUM"))

                addmask = const_pool.tile([128, NB, BW], F32)
                nc.gpsimd.memset(addmask[:], 0.0)
                for k in range(NB):
                    nc.gpsimd.affine_select(
                        out=addmask[:, k, :], in_=addmask[:, k, :],
                        pattern=[[1, BW]], compare_op=mybir.AluOpType.is_ge,
                        fill=-1e30, base=-128 * k, channel_multiplier=-1,
                    )
                ones_row = const_pool.tile([128, 2], F32R)
                nc.vector.memset(ones_row[:].bitcast(F32), 1.0)
                ones_col = const_pool.tile([1, 128], F32)
                nc.vector.memset(ones_col[:], 1.0)

                wout_sb = wout_pool.tile([128, HPC, C], F32R)
                for h in range(HPC):
                    nc.sync.dma_start(wout_sb[:, h], wout[:, h])

                for b in range(NB):
                    aot = aot_pool.tile([128, HPC, BW], F32R)
                    for h in range(HPC):
                        qt = qt_pool.tile([128, BW], F32R)
                        nc.sync.dma_start(qt[:], qk_dr[h][b][:])
                        nj = 4 * b + 4
                        ps_o = psd_o.tile([128, BW], F32)
                        ps_n = psd_n.tile([2, BW], F32)
                        for j in range(nj):
                            kt = kt_pool.tile([128, 128], F32R)
                            nc.sync.dma_start(kt[:], qk_dr[HPC + h][j // 4][:, (j % 4) * 128:(j % 4 + 1) * 128])
                            ps_s = psd_s.tile([128, BW], F32, tag="ps512")
                            nc.tensor.matmul(ps_s[:], kt[:], qt[:], start=True, stop=True)
                            if j >= 4 * b:
                                nc.vector.tensor_add(ps_s[:], ps_s[:], addmask[:, j - 4 * b])
                            at = at_pool.tile([128, BW], F32R)
                            nc.scalar.activation(
                                at[:], ps_s[:], mybir.ActivationFunctionType.Exp,
                                scale=SCALE,
                            )
                            nc.tensor.matmul(ps_o[:], vp[j][:, h], at[:],
                                             start=(j == 0), stop=(j == nj - 1))
                            nc.tensor.matmul(ps_n[:], ones_row[:], at[:],
                                             start=(j == 0), stop=(j == nj - 1))
                        rec = rec_pool.tile([1, BW], F32)
                        nc.vector.reciprocal(rec[:], ps_n[0:1, :])
                        ps_b = psd_s.tile([128, BW], F32, tag="ps512")
                        nc.tensor.matmul(ps_b[:], ones_col[:], rec[:], start=True, stop=True)
                        bc = bc_pool.tile([128, BW], F32)
                        nc.scalar.activation(bc[:], ps_b[:], mybir.ActivationFunctionType.Copy)
                        nc.vector.tensor_mul(aot[:, h], ps_o[:], bc[:])
                    for il in range(4):
                        for cb in range(4):
                            ps_f = psd_s.tile([128, BW], F32, tag="ps512")
                            for h in range(HPC):
                                nc.tensor.matmul(
                                    ps_f[:], aot[:, h, il * 128:(il + 1) * 128],
                                    wout_sb[:, h, cb * BW:(cb + 1) * BW],
                                    start=(h == 0), stop=(h == HPC - 1),
                                )
                            oc = oc_pool.tile([128, BW], F32)
                            nc.vector.tensor_copy(oc[:], ps_f[:])
                            nc.sync.dma_start(
                                out[(4 * b + il) * 128:(4 * b + il + 1) * 128,
                                    cb * BW:(cb + 1) * BW], oc[:],
                            )

    nc.compile()
    return nc


_NC = None


def _get_nc():
    global _NC
    if _NC is None:
        _NC = build_nc()
    return _NC


def kernel(x, mask, Wqkv, Wout, _trace=False):
    assert x.shape == (B, T, C) and Wqkv.shape == (C, 3 * C) and Wout.shape == (C, C)
    nc = _get_nc()

    xt = [round_fp32r(np.ascontiguousarray(x[b].T)) for b in range(B)]
    in_maps = []
    for c in range(8):
        b, g = c // 4, c % 4
        h0 = g * HPC * DH          # column offset of this core's heads
        wqk_c = round_fp32r(np.ascontiguousarray(
            np.concatenate([Wqkv[:, h0:h0 + HPC * DH],
                            Wqkv[:, C + h0:C + h0 + HPC * DH]], axis=1)))
        wv_c = round_fp32r(np.ascontiguousarray(Wqkv[:, 2 * C + h0:2 * C + h0 + HPC * DH]))
        wout_c = round_fp32r(np.ascontiguousarray(Wout[h0:h0 + HPC * DH, :]))
        in_maps.append({"xt": xt[b], "wqk": wqk_c, "wv": wv_c, "wout": wout_c})

    kwargs = {}
    if _trace:
        import os
        kwargs = dict(trace=True, tmpdir=os.environ.get("KERNEL_TRACE_DIR"))
    res = run_bass_kernel_spmd(nc, in_maps, core_ids=list(range(8)), **kwargs)

    outs = np.zeros((B, T, C), dtype=np.float64)
    for c in range(8):
        outs[c // 4] += res.results[c]["out"].astype(np.float64)
    result = outs.astype(np.float32)
    if _trace:
        return result, res
    return result


# revision 14
# speedup vs baseline: 1.2143x; 1.2143x over previous
"""Multi-head causal attention (B=2, T=2048, C=2048, 16 heads, fp32) on 8
Trainium2 NeuronCores.

Sharding: data-parallel over batch (2) x tensor-parallel over heads
(4 heads/core).  Core c handles batch c//4, heads 4*(c%4)..4*(c%4)+3.
Each core computes q/k/v projections for its heads, causal softmax
attention, and a partial output projection (its heads' rows of Wout);
the host sums the 4 partials per batch.

All big matmuls run in float32r (fp32 with 11-bit mantissa): products of
fp32r values accumulate exactly in fp32 PSUM, so the only precision loss
is the input rounding (~1.2e-4 relative).

Device program per core:
  Phase A: stream x^T in 256-wide t-slabs against resident Wqk/Wv.
    q^T,k^T (head-dim on partitions) bounce through DRAM scratch tiles
    (one per (row, tq-block) for fine-grained deps); v ([t,d] layout)
    stays resident in SBUF as one tile per t-tile.
  Phase D, per 512-wide tq-block b, per head:
    for each tk-tile j<=diag: scoresT[tk,tq] = kT_j^T @ qT_b on PE;
    additive -1e30 causal mask on diagonal-crossing tiles (DVE, static
    mask tiles built once via affine_select); exp via ACT
    (scale=1/sqrt(128) folded in) -> attnT_j in fp32r; then
      outT[d,tq]   += v_j^T   @ attnT_j   (n=512, full-rate fp32r)
      denom[1,tq]  += ones^T  @ attnT_j   (softmax denominators)
    normalize: DVE reciprocal of denom -> broadcast over partitions via
    a K=1 fp32 matmul -> attn_outT = outT * recip (DVE), directly in the
    [d, tq] layout the output projection needs (no transposes).
  Output projection per block: out[t,c] += attn_outT_h^T @ Wout_h.
"""

import numpy as np

import concourse.bass as bass
import concourse.tile as tile
from concourse import bacc, mybir
from concourse.bass_utils import run_bass_kernel_spmd

B, T, C = 2, 2048, 2048
H, DH = 16, 128
HPC = 4            # heads per core
KO = C // 128      # 16 contraction tiles
NSLAB = 8          # 256-wide t slabs in phase A
SLAB = T // NSLAB  # 256
NB = 4             # 512-wide tq blocks in phase D
BW = T // NB       # 512
NT = T // 128      # 16 t tiles
SCALE = DH ** -0.5
F32 = mybir.dt.float32
F32R = mybir.dt.float32r


def round_fp32r(x: np.ndarray) -> np.ndarray:
    """Round fp32 to fp32r (11-bit mantissa) the way the PE expects."""
    u = x.astype(np.float32, copy=True).view(np.uint32).astype(np.uint64)
    u = ((u + 0x1000) >> 13) << 13
    return (u & 0xFFFFFFFF).astype(np.uint32).view(np.float32)


def build_nc():
    nc = bacc.Bacc("TRN2", target_bir_lowering=False, debug=False, num_devices=8)
    xt_d = nc.dram_tensor("xt", [C, T], F32R, kind="ExternalInput")
    wqk_d = nc.dram_tensor("wqk", [C, 2 * HPC * DH], F32R, kind="ExternalInput")
    wv_d = nc.dram_tensor("wv", [C, HPC * DH], F32R, kind="ExternalInput")
    wout_d = nc.dram_tensor("wout", [HPC * DH, C], F32R, kind="ExternalInput")
    out_d = nc.dram_tensor("out", [T, C], F32, kind="ExternalOutput")

    xt = xt_d.ap().rearrange("(ko p) t -> p ko t", p=128)
    wqk = wqk_d.ap().rearrange("(ko p) m -> p ko m", p=128)
    wv = wv_d.ap().rearrange("(ko p) m -> p ko m", p=128)
    wout = wout_d.ap().rearrange("(h p) c -> p h c", p=128)
    out = out_d.ap()

    with tile.TileContext(nc) as tc:
        from contextlib import ExitStack

        with ExitStack() as top:
            # ---- persistent across phases ----
            vp_pool = top.enter_context(tc.tile_pool(name="vp", bufs=NT))
            dram_pool = top.enter_context(tc.tile_pool(name="dram", bufs=1, space="DRAM"))

            # v resident, one tile per t-tile: [tk within tile, head, d]
            vp = [vp_pool.tile([128, HPC, DH], F32R, name=f"vp{j}", tag="vp")
                  for j in range(NT)]
            # q^T/k^T DRAM scratch, one tile per (row co, tq block):
            # co 0..3 = q heads, 4..7 = k heads
            qk_dr = [[dram_pool.tile([128, BW], F32R, name=f"qk{co}_{b}")
                      for b in range(NB)] for co in range(2 * HPC)]

            # ================= Phase A: projections =================
            with ExitStack() as pa:
                wqk_pool = pa.enter_context(tc.tile_pool(name="wqk", bufs=1))
                wv_pool = pa.enter_context(tc.tile_pool(name="wv", bufs=1))
                slab_pool = pa.enter_context(tc.tile_pool(name="slab", bufs=2))
                bounce_pool = pa.enter_context(tc.tile_pool(name="bounce", bufs=3))
                psa_qk = pa.enter_context(tc.tile_pool(name="psa_qk", bufs=2, space="PSUM"))
                psa_v = pa.enter_context(tc.tile_pool(name="psa_v", bufs=2, space="PSUM"))

                wqk_sb = wqk_pool.tile([128, KO, 2 * HPC * DH], F32R)
                wv_sb = wv_pool.tile([128, KO, HPC * DH], F32R)

                for s in range(NSLAB):
                    slab = slab_pool.tile([128, KO, SLAB], F32R)
                    for ko in range(KO):
                        if s == 0:
                            # interleave weight/x loads so the first psum
                            # accumulation chain unblocks per-ko
                            nc.sync.dma_start(wqk_sb[:, ko], wqk[:, ko])
                        nc.sync.dma_start(slab[:, ko], xt[:, ko, s * SLAB:(s + 1) * SLAB])
                    if s == 0:
                        for ko in range(KO):
                            nc.sync.dma_start(wv_sb[:, ko], wv[:, ko])
                    for co in range(2 * HPC):
                        ps = psa_qk.tile([128, SLAB], F32)
                        for ko in range(KO):
                            nc.tensor.matmul(
                                ps[:], wqk_sb[:, ko, co * 128:(co + 1) * 128],
                                slab[:, ko], start=(ko == 0), stop=(ko == KO - 1),
                            )
                        bt = bounce_pool.tile([128, SLAB], F32R)
                        nc.vector.tensor_copy(bt[:], ps[:])
                        blk, off = (s * SLAB) // BW, (s * SLAB) % BW
                        nc.sync.dma_start(qk_dr[co][blk][:, off:off + SLAB], bt[:])
                    for tt in range(SLAB // 128):
                        ps = psa_v.tile([128, HPC * DH], F32)
                        for ko in range(KO):
                            nc.tensor.matmul(
                                ps[:], slab[:, ko, tt * 128:(tt + 1) * 128],
                                wv_sb[:, ko], start=(ko == 0), stop=(ko == KO - 1),
                            )
                        ti = s * (SLAB // 128) + tt
                        for h in range(HPC):
                            nc.vector.tensor_copy(vp[ti][:, h], ps[:, h * 128:(h + 1) * 128])

            # ============ Phase D: attention + out projection ============
            with ExitStack() as pd:
                const_pool = pd.enter_context(tc.tile_pool(name="const", bufs=1))
                wout_pool = pd.enter_context(tc.tile_pool(name="wout", bufs=1))
                kt_pool = pd.enter_context(tc.tile_pool(name="kt", bufs=4))
                qt_pool = pd.enter_context(tc.tile_pool(name="qt", bufs=2))
                at_pool = pd.enter_context(tc.tile_pool(name="at", bufs=3))
                rec_pool = pd.enter_context(tc.tile_pool(name="rec", bufs=2))
                aot_pool = pd.enter_context(tc.tile_pool(name="aot", bufs=2))
                oc_pool = pd.enter_context(tc.tile_pool(name="oc", bufs=3))
                psd_s = pd.enter_context(tc.tile_pool(name="psd_s", bufs=2, space="PSUM"))
                psd_o = pd.enter_context(tc.tile_pool(name="psd_o", bufs=2, space="PSUM"))
                psd_n = pd.enter_context(tc.tile_pool(name="psd_n", bufs=2, space="PSUM"))

                addmask = const_pool.tile([128, NB, BW], F32)
                nc.gpsimd.memset(addmask[:], 0.0)
                for k in range(NB):
                    nc.gpsimd.affine_select(
                        out=addmask[:, k, :], in_=addmask[:, k, :],
                        pattern=[[1, BW]], compare_op=mybir.AluOpType.is_ge,
                        fill=-1e30, base=-128 * k, channel_multiplier=-1,
                    )
                ones_mat = const_pool.tile([128, 128], F32R)
                nc.vector.memset(ones_mat[:].bitcast(F32), 1.0)

                wout_sb = wout_pool.tile([128, HPC, C], F32R)
                for h in range(HPC):
                    nc.sync.dma_start(wout_sb[:, h], wout[:, h])

                for b in range(NB):
                    aot = aot_pool.tile([128, HPC, BW], F32R)
                    for h in range(HPC):
                        qt = qt_pool.tile([128, BW], F32R)
                        nc.sync.dma_start(qt[:], qk_dr[h][b][:])
                        nj = 4 * b + 4
                        ps_o = psd_o.tile([128, BW], F32)
                        ps_n = psd_n.tile([128, BW], F32)
                        for j in range(nj):
                            kt = kt_pool.tile([128, 128], F32R)
                            nc.sync.dma_start(kt[:], qk_dr[HPC + h][j // 4][:, (j % 4) * 128:(j % 4 + 1) * 128])
                            ps_s = psd_s.tile([128, BW], F32, tag="ps512")
                            nc.tensor.matmul(ps_s[:], kt[:], qt[:], start=True, stop=True)
                            if j >= 4 * b:
                                nc.vector.tensor_add(ps_s[:], ps_s[:], addmask[:, j - 4 * b])
                            at = at_pool.tile([128, BW], F32R)
                            nc.scalar.activation(
                                at[:], ps_s[:], mybir.ActivationFunctionType.Exp,
                                scale=SCALE,
                            )
                            nc.tensor.matmul(ps_o[:], vp[j][:, h], at[:],
                                             start=(j == 0), stop=(j == nj - 1))
                            nc.tensor.matmul(ps_n[:], ones_mat[:], at[:],
                                             start=(j == 0), stop=(j == nj - 1))
                        rec = rec_pool.tile([128, BW], F32)
                        nc.vector.reciprocal(rec[:], ps_n[:])
                        nc.vector.tensor_mul(aot[:, h], ps_o[:], rec[:])
                    for il in range(4):
                        for cb in range(4):
                            ps_f = psd_s.tile([128, BW], F32, tag="ps512")
                            for h in range(HPC):
                                nc.tensor.matmul(
                                    ps_f[:], aot[:, h, il * 128:(il + 1) * 128],
                                    wout_sb[:, h, cb * BW:(cb + 1) * BW],
                                    start=(h == 0), stop=(h == HPC - 1),
                                )
                            oc = oc_pool.tile([128, BW], F32)
                            nc.vector.tensor_copy(oc[:], ps_f[:])
                            nc.sync.dma_start(
                                out[(4 * b + il) * 128:(4 * b + il + 1) * 128,
                                    cb * BW:(cb + 1) * BW], oc[:],
                            )

    nc.compile()
    return nc


_NC = None


def _get_nc():
    global _NC
    if _NC is None:
        _NC = build_nc()
    return _NC


def kernel(x, mask, Wqkv, Wout, _trace=False):
    assert x.shape == (B, T, C) and Wqkv.shape == (C, 3 * C) and Wout.shape == (C, C)
    nc = _get_nc()

    xt = [round_fp32r(np.ascontiguousarray(x[b].T)) for b in range(B)]
    in_maps = []
    for c in range(8):
        b, g = c // 4, c % 4
        h0 = g * HPC * DH          # column offset of this core's heads
        wqk_c = round_fp32r(np.ascontiguousarray(
            np.concatenate([Wqkv[:, h0:h0 + HPC * DH],
                            Wqkv[:, C + h0:C + h0 + HPC * DH]], axis=1)))
        wv_c = round_fp32r(np.ascontiguousarray(Wqkv[:, 2 * C + h0:2 * C + h0 + HPC * DH]))
        wout_c = round_fp32r(np.ascontiguousarray(Wout[h0:h0 + HPC * DH, :]))
        in_maps.append({"xt": xt[b], "wqk": wqk_c, "wv": wv_c, "wout": wout_c})

    kwargs = {}
    if _trace:
        import os
        kwargs = dict(trace=True, tmpdir=os.environ.get("KERNEL_TRACE_DIR"))
    res = run_bass_kernel_spmd(nc, in_maps, core_ids=list(range(8)), **kwargs)

    outs = np.zeros((B, T, C), dtype=np.float64)
    for c in range(8):
        outs[c // 4] += res.results[c]["out"].astype(np.float64)
    result = outs.astype(np.float32)
    if _trace:
        return result, res
    return result


# revision 19
# speedup vs baseline: 1.2287x; 1.0118x over previous
"""Multi-head causal attention (B=2, T=2048, C=2048, 16 heads, fp32) on 8
Trainium2 NeuronCores.

Sharding: data-parallel over batch (2) x tensor-parallel over heads
(4 heads/core).  Core c handles batch c//4, heads 4*(c%4)..4*(c%4)+3.
Each core computes q/k/v projections for its heads, causal softmax
attention, and a partial output projection (its heads' rows of Wout);
the host sums the 4 partials per batch.

All big matmuls run in float32r (fp32 with 11-bit mantissa): products of
fp32r values accumulate exactly in fp32 PSUM, so the only precision loss
is the input rounding (~1.2e-4 relative).

Device program per core:
  Phase A: stream x^T in 256-wide t-slabs against resident Wqk/Wv.
    q^T,k^T (head-dim on partitions) bounce through DRAM scratch tiles
    (one per (row, tq-block) for fine-grained deps); v ([t,d] layout)
    stays resident in SBUF as one tile per t-tile.
  Phase D, per 512-wide tq-block b, per head:
    for each tk-tile j<=diag: scoresT[tk,tq] = kT_j^T @ qT_b on PE;
    additive -1e30 causal mask on diagonal-crossing tiles (DVE, static
    mask tiles built once via affine_select); exp via ACT
    (scale=1/sqrt(128) folded in) -> attnT_j in fp32r; then
      outT[d,tq]   += v_j^T   @ attnT_j   (n=512, full-rate fp32r)
      denom[1,tq]  += ones^T  @ attnT_j   (softmax denominators)
    normalize: DVE reciprocal of denom -> broadcast over partitions via
    a K=1 fp32 matmul -> attn_outT = outT * recip (DVE), directly in the
    [d, tq] layout the output projection needs (no transposes).
  Output projection per block: out[t,c] += attn_outT_h^T @ Wout_h.
"""

import numpy as np

import concourse.bass as bass
import concourse.tile as tile
from concourse import bacc, mybir
from concourse.bass_utils import run_bass_kernel_spmd

B, T, C = 2, 2048, 2048
H, DH = 16, 128
HPC = 4            # heads per core
KO = C // 128      # 16 contraction tiles
NSLAB = 4          # 512-wide t slabs in phase A
SLAB = T // NSLAB  # 512
NB = 4             # 512-wide tq blocks in phase D
BW = T // NB       # 512
NT = T // 128      # 16 t tiles
SCALE = DH ** -0.5
F32 = mybir.dt.float32
F32R = mybir.dt.float32r


def round_fp32r(x: np.ndarray) -> np.ndarray:
    """Round fp32 to fp32r (11-bit mantissa) the way the PE expects."""
    u = x.astype(np.float32, copy=True).view(np.uint32).astype(np.uint64)
    u = ((u + 0x1000) >> 13) << 13
    return (u & 0xFFFFFFFF).astype(np.uint32).view(np.float32)


def build_nc():
    nc = bacc.Bacc("TRN2", target_bir_lowering=False, debug=False, num_devices=8)
    xt_d = nc.dram_tensor("xt", [C, T], F32R, kind="ExternalInput")
    wqk_d = nc.dram_tensor("wqk", [C, 2 * HPC * DH], F32R, kind="ExternalInput")
    wv_d = nc.dram_tensor("wv", [C, HPC * DH], F32R, kind="ExternalInput")
    wout_d = nc.dram_tensor("wout", [HPC * DH, C], F32R, kind="ExternalInput")
    out_d = nc.dram_tensor("out", [T, C], F32, kind="ExternalOutput")

    xt = xt_d.ap().rearrange("(ko p) t -> p ko t", p=128)
    wqk = wqk_d.ap().rearrange("(ko p) m -> p ko m", p=128)
    wv = wv_d.ap().rearrange("(ko p) m -> p ko m", p=128)
    wout = wout_d.ap().rearrange("(h p) c -> p h c", p=128)
    out = out_d.ap()

    with tile.TileContext(nc) as tc:
        from contextlib import ExitStack

        with ExitStack() as top:
            # ---- persistent across phases ----
            vp_pool = top.enter_context(tc.tile_pool(name="vp", bufs=NT))
            dram_pool = top.enter_context(tc.tile_pool(name="dram", bufs=1, space="DRAM"))

            # v resident, one tile per t-tile: [tk within tile, head, d]
            vp = [vp_pool.tile([128, HPC, DH], F32R, name=f"vp{j}", tag="vp")
                  for j in range(NT)]
            # q^T/k^T DRAM scratch, one tile per (row co, tq block):
            # co 0..3 = q heads, 4..7 = k heads
            qk_dr = [[dram_pool.tile([128, BW], F32R, name=f"qk{co}_{b}")
                      for b in range(NB)] for co in range(2 * HPC)]

            # ================= Phase A: projections =================
            with ExitStack() as pa:
                wqk_pool = pa.enter_context(tc.tile_pool(name="wqk", bufs=1))
                wv_pool = pa.enter_context(tc.tile_pool(name="wv", bufs=1))
                slab_pool = pa.enter_context(tc.tile_pool(name="slab", bufs=2))
                bounce_pool = pa.enter_context(tc.tile_pool(name="bounce", bufs=3))
                psa_qk = pa.enter_context(tc.tile_pool(name="psa_qk", bufs=2, space="PSUM"))
                psa_v = pa.enter_context(tc.tile_pool(name="psa_v", bufs=2, space="PSUM"))

                wqk_sb = wqk_pool.tile([128, KO, 2 * HPC * DH], F32R)
                wv_sb = wv_pool.tile([128, KO, HPC * DH], F32R)

                for s in range(NSLAB):
                    slab = slab_pool.tile([128, KO, SLAB], F32R)
                    for ko in range(KO):
                        if s == 0:
                            # interleave weight/x loads so the first psum
                            # accumulation chain unblocks per-ko
                            nc.sync.dma_start(wqk_sb[:, ko], wqk[:, ko])
                        nc.sync.dma_start(slab[:, ko], xt[:, ko, s * SLAB:(s + 1) * SLAB])
                    if s == 0:
                        for ko in range(KO):
                            nc.sync.dma_start(wv_sb[:, ko], wv[:, ko])
                    for co in range(2 * HPC):
                        ps = psa_qk.tile([128, SLAB], F32)
                        for ko in range(KO):
                            nc.tensor.matmul(
                                ps[:], wqk_sb[:, ko, co * 128:(co + 1) * 128],
                                slab[:, ko], start=(ko == 0), stop=(ko == KO - 1),
                            )
                        bt = bounce_pool.tile([128, SLAB], F32R)
                        nc.vector.tensor_copy(bt[:], ps[:])
                        nc.sync.dma_start(qk_dr[co][s][:], bt[:])
                    for tt in range(SLAB // 128):
                        ps = psa_v.tile([128, HPC * DH], F32)
                        for ko in range(KO):
                            nc.tensor.matmul(
                                ps[:], slab[:, ko, tt * 128:(tt + 1) * 128],
                                wv_sb[:, ko], start=(ko == 0), stop=(ko == KO - 1),
                            )
                        ti = s * (SLAB // 128) + tt
                        for h in range(HPC):
                            nc.vector.tensor_copy(vp[ti][:, h], ps[:, h * 128:(h + 1) * 128])

            # ============ Phase D: attention + out projection ============
            with ExitStack() as pd:
                const_pool = pd.enter_context(tc.tile_pool(name="const", bufs=1))
                wout_pool = pd.enter_context(tc.tile_pool(name="wout", bufs=1))
                kt_pool = pd.enter_context(tc.tile_pool(name="kt", bufs=6))
                qt_pool = pd.enter_context(tc.tile_pool(name="qt", bufs=2))
                at_pool = pd.enter_context(tc.tile_pool(name="at", bufs=3))
                rec_pool = pd.enter_context(tc.tile_pool(name="rec", bufs=2))
                aot_pool = pd.enter_context(tc.tile_pool(name="aot", bufs=2))
                oc_pool = pd.enter_context(tc.tile_pool(name="oc", bufs=3))
                psd_s = pd.enter_context(tc.tile_pool(name="psd_s", bufs=2, space="PSUM"))
                psd_o = pd.enter_context(tc.tile_pool(name="psd_o", bufs=2, space="PSUM"))
                psd_n = pd.enter_context(tc.tile_pool(name="psd_n", bufs=2, space="PSUM"))

                addmask = const_pool.tile([128, NB, BW], F32)
                nc.gpsimd.memset(addmask[:], 0.0)
                for k in range(NB):
                    nc.gpsimd.affine_select(
                        out=addmask[:, k, :], in_=addmask[:, k, :],
                        pattern=[[1, BW]], compare_op=mybir.AluOpType.is_ge,
                        fill=-1e30, base=-128 * k, channel_multiplier=-1,
                    )
                ones_mat = const_pool.tile([128, 128], F32R)
                nc.vector.memset(ones_mat[:].bitcast(F32), 1.0)

                wout_sb = wout_pool.tile([128, HPC, C], F32R)
                for h in range(HPC):
                    nc.sync.dma_start(wout_sb[:, h], wout[:, h])

                def final_proj(bb, aot_bb):
                    for il in range(4):
                        for cb in range(4):
                            ps_f = psd_s.tile([128, BW], F32, tag="ps512", name="ps_f")
                            for h in range(HPC):
                                nc.tensor.matmul(
                                    ps_f[:], aot_bb[:, h, il * 128:(il + 1) * 128],
                                    wout_sb[:, h, cb * BW:(cb + 1) * BW],
                                    start=(h == 0), stop=(h == HPC - 1),
                                )
                            oc = oc_pool.tile([128, BW], F32)
                            nc.vector.tensor_copy(oc[:], ps_f[:])
                            nc.sync.dma_start(
                                out[(4 * bb + il) * 128:(4 * bb + il + 1) * 128,
                                    cb * BW:(cb + 1) * BW], oc[:],
                            )

                aots = []
                for b in range(NB):
                    aot = aot_pool.tile([128, HPC, BW], F32R)
                    aots.append(aot)
                    for h in range(HPC):
                        qt = qt_pool.tile([128, BW], F32R)
                        nc.sync.dma_start(qt[:], qk_dr[h][b][:])
                        kts = []
                        for jb in range(b + 1):
                            kt = kt_pool.tile([128, BW], F32R, tag="kt", name=f"kt{jb}")
                            nc.sync.dma_start(kt[:], qk_dr[HPC + h][jb][:])
                            kts.append(kt)
                        nj = 4 * b + 4
                        ps_o = psd_o.tile([128, BW], F32)
                        ps_n = psd_n.tile([128, BW], F32)
                        pending = None
                        for j in range(nj):
                            ps_s = psd_s.tile([128, BW], F32, tag="ps512", name="ps_s")
                            nc.tensor.matmul(
                                ps_s[:], kts[j // 4][:, (j % 4) * 128:(j % 4 + 1) * 128],
                                qt[:], start=True, stop=True)
                            if j >= 4 * b:
                                nc.vector.tensor_add(ps_s[:], ps_s[:], addmask[:, j - 4 * b])
                            at = at_pool.tile([128, BW], F32R)
                            nc.scalar.activation(
                                at[:], ps_s[:], mybir.ActivationFunctionType.Exp,
                                scale=SCALE,
                            )
                            if pending is not None:
                                pat, pj = pending
                                nc.tensor.matmul(ps_o[:], vp[pj][:, h], pat[:],
                                                 start=(pj == 0), stop=False)
                                nc.tensor.matmul(ps_n[:], ones_mat[:], pat[:],
                                                 start=(pj == 0), stop=False)
                            pending = (at, j)
                        pat, pj = pending
                        nc.tensor.matmul(ps_o[:], vp[pj][:, h], pat[:],
                                         start=(pj == 0), stop=True)
                        nc.tensor.matmul(ps_n[:], ones_mat[:], pat[:],
                                         start=(pj == 0), stop=True)
                        rec = rec_pool.tile([128, BW], F32)
                        nc.vector.reciprocal(rec[:], ps_n[:])
                        nc.vector.tensor_mul(aot[:, h], ps_o[:], rec[:])
                        if h == 0 and b > 0:
                            final_proj(b - 1, aots[b - 1])
                final_proj(NB - 1, aots[NB - 1])

    nc.compile()
    return nc


_NC = None


def _get_nc():
    global _NC
    if _NC is None:
        _NC = build_nc()
    return _NC


def kernel(x, mask, Wqkv, Wout, _trace=False):
    assert x.shape == (B, T, C) and Wqkv.shape == (C, 3 * C) and Wout.shape == (C, C)
    nc = _get_nc()

    xt = [round_fp32r(np.ascontiguousarray(x[b].T)) for b in range(B)]
    in_maps = []
    for c in range(8):
        b, g = c // 4, c % 4
        h0 = g * HPC * DH          # column offset of this core's heads
        wqk_c = round_fp32r(np.ascontiguousarray(
            np.concatenate([Wqkv[:, h0:h0 + HPC * DH],
                            Wqkv[:, C + h0:C + h0 + HPC * DH]], axis=1)))
        wv_c = round_fp32r(np.ascontiguousarray(Wqkv[:, 2 * C + h0:2 * C + h0 + HPC * DH]))
        wout_c = round_fp32r(np.ascontiguousarray(Wout[h0:h0 + HPC * DH, :]))
        in_maps.append({"xt": xt[b], "wqk": wqk_c, "wv": wv_c, "wout": wout_c})

    kwargs = {}
    if _trace:
        import os
        kwargs = dict(trace=True, tmpdir=os.environ.get("KERNEL_TRACE_DIR"))
    res = run_bass_kernel_spmd(nc, in_maps, core_ids=list(range(8)), **kwargs)

    outs = np.zeros((B, T, C), dtype=np.float64)
    for c in range(8):
        outs[c // 4] += res.results[c]["out"].astype(np.float64)
    result = outs.astype(np.float32)
    if _trace:
        return result, res
    return result
